# revision 18
# baseline (speedup 1.0000x reference)
"""Trainium2 Bass kernel for a single AttnDecoderRNN step, SPMD over 8 NeuronCores.

Computation (matching the jax reference):
    x       = emb[input_seq]                       [B, H]
    h_new   = GRU_step(x, last_hidden)             [B, H]
    scores  = h_new . (attn_w @ enc + attn_b)      [B, S]
    attn_w8 = softmax(scores)                      [B, S]
    context = attn_w8 @ enc                        [B, H]
    concat  = tanh([h_new, context] @ concat_w.T)  [B, H]
    logits  = concat @ out_w.T + out_b             [B, V]

Key algebraic restructurings (all mathematically exact):
  * scores[b,s] = (h_new[b] @ attn_w) . enc[s,b] + h_new[b].attn_b: project the
    query once instead of projecting every encoder state (275 GFLOP -> 0.13).
  * The h_new.attn_b term is constant over s, and softmax is shift-invariant,
    so attn_b drops out of the output entirely.
  * softmax without max-subtraction: scores here are O(25) << 88, so exp() is
    safe in fp32, which lets scores and context accumulate in one fused pass
    over enc (no second read of the 512MB encoder tensor).

Sharding over 8 cores:
  * GRU gates h-sharded (1/8 of the 3H gate rows per core); an AllToAll swaps
    h-slices for batch-slices so each core holds h_newT for its own 8 batches.
  * Attention/concat batch-sharded (8 batches per core); enc streamed once.
  * Output projection V-sharded (4000 vocab columns per core) after an
    AllGather of the [64, 1024] concat output.
"""

import os
import numpy as np

H = 1024
V = 32000
B = 64
S = 2048
N = 8            # cores
BL = B // N      # 8  batches per core
HL = H // N      # 128 hidden rows per core (GRU shard)
VL = V // N      # 4000 vocab cols per core
VC = 500         # vocab chunk per PSUM tile (VL / VC = 8)
SC = S // 128    # 16 s-chunks of 128
F32 = None       # set after mybir import

_cache = {}


def _build():
    import concourse.bacc as bacc
    import concourse.bass as bass
    import concourse.mybir as mybir
    import concourse.tile as tile

    f32 = mybir.dt.float32
    i32 = mybir.dt.int32
    AF = mybir.ActivationFunctionType
    OP = mybir.AluOpType

    nc = bacc.Bacc("TRN2", target_bir_lowering=False, debug=False, num_devices=N)

    # ---- per-core DRAM inputs -------------------------------------------
    seq_idx = nc.dram_tensor("seq_idx", [B, 1], i32, kind="ExternalInput").ap()
    emb_t = nc.dram_tensor("emb", [V, H], f32, kind="ExternalInput").ap()
    hprev = nc.dram_tensor("hprev", [B, H], f32, kind="ExternalInput").ap()
    hprev_sl = nc.dram_tensor("hprev_sl", [B, HL], f32, kind="ExternalInput").ap()
    enc_l = nc.dram_tensor("enc_l", [S, BL, H], f32, kind="ExternalInput").ap()
    wihT_l = nc.dram_tensor("wihT_l", [H, 3 * HL], f32, kind="ExternalInput").ap()
    whhT_l = nc.dram_tensor("whhT_l", [H, 3 * HL], f32, kind="ExternalInput").ap()
    bihT_l = nc.dram_tensor("bihT_l", [HL, 3], f32, kind="ExternalInput").ap()
    bhhT_l = nc.dram_tensor("bhhT_l", [HL, 3], f32, kind="ExternalInput").ap()
    attn_wt = nc.dram_tensor("attn_w", [H, H], f32, kind="ExternalInput").ap()
    concat_wT = nc.dram_tensor("concat_wT", [2 * H, H], f32, kind="ExternalInput").ap()
    concat_b_r = nc.dram_tensor("concat_b_r", [1, H], f32, kind="ExternalInput").ap()
    out_wT_l = nc.dram_tensor("out_wT_l", [H, VL], f32, kind="ExternalInput").ap()
    out_b_l = nc.dram_tensor("out_b_l", [1, VL], f32, kind="ExternalInput").ap()
    ident = nc.dram_tensor("ident", [128, 128], f32, kind="ExternalInput").ap()

    # ---- per-core DRAM outputs ------------------------------------------
    logits_l = nc.dram_tensor("logits_l", [B, VL], f32, kind="ExternalOutput").ap()
    hnew_l = nc.dram_tensor("hnew_l", [B, HL], f32, kind="ExternalOutput").ap()
    attnw_l = nc.dram_tensor("attnw_l", [BL, S], f32, kind="ExternalOutput").ap()
    debug = bool(os.environ.get("ATTN_KERNEL_DEBUG"))
    if debug:
        dbg_ctx = nc.dram_tensor("dbg_ctx", [128, 64], f32, kind="ExternalOutput").ap()
        dbg_co = nc.dram_tensor("dbg_co", [BL, H], f32, kind="ExternalOutput").ap()
        dbg_cof = nc.dram_tensor("dbg_cof", [B, H], f32, kind="ExternalOutput").ap()
        dbg_coT = nc.dram_tensor("dbg_coT", [128, 8 * B], f32, kind="ExternalOutput").ap()

    rg = [list(range(N))]

    with tile.TileContext(nc) as tc:
        with (
            tc.tile_pool(name="const", bufs=1) as cst,
            tc.tile_pool(name="dram", bufs=1, space="DRAM") as dramp,
        ):
            identity = cst.tile([128, 128], f32)
            nc.sync.dma_start(identity[:], ident[:])
            ones_col = cst.tile([128, 1], f32)
            nc.vector.memset(ones_col[:], 1.0)

            # =========================== Phase 1: GRU ====================
            with (
                tc.tile_pool(name="ph1", bufs=1) as p1,
                tc.tile_pool(name="ph1ps", bufs=1, space="PSUM") as p1ps,
                tc.tile_pool(name="tps", bufs=1, space="PSUM") as tps,
            ):
                # embedding gather: x = emb[input_seq]  -> [64, 1024]
                idx_sb = p1.tile([B, 1], i32)
                nc.sync.dma_start(idx_sb[:], seq_idx[:])
                x_sb = p1.tile([B, H], f32)
                nc.gpsimd.indirect_dma_start(
                    out=x_sb[:],
                    out_offset=None,
                    in_=emb_t[:, :],
                    in_offset=bass.IndirectOffsetOnAxis(ap=idx_sb[:, :1], axis=0),
                )

                hp_sb = p1.tile([B, H], f32)
                nc.sync.dma_start(hp_sb[:], hprev[:])
                hpsl_sb = p1.tile([B, HL], f32)
                nc.sync.dma_start(hpsl_sb[:], hprev_sl[:])

                # transpose x and h_prev into [H(part), B] chunk layout
                xT_sb = p1.tile([128, 8 * B], f32)
                hpT_sb = p1.tile([128, 8 * B], f32)
                for hc in range(8):
                    pt = tps.tile([128, B], f32, tag="tp")
                    nc.tensor.transpose(pt[:], x_sb[:, hc * 128:(hc + 1) * 128],
                                        identity[0:B, 0:B])
                    nc.scalar.copy(xT_sb[:, hc * B:(hc + 1) * B], pt[:])
                    pt2 = tps.tile([128, B], f32, tag="tp")
                    nc.tensor.transpose(pt2[:], hp_sb[:, hc * 128:(hc + 1) * 128],
                                        identity[0:B, 0:B])
                    nc.scalar.copy(hpT_sb[:, hc * B:(hc + 1) * B], pt2[:])
                hpslT_sb = p1.tile([128, B], f32)
                pt3 = tps.tile([128, B], f32, tag="tp")
                nc.tensor.transpose(pt3[:], hpsl_sb[:, :], identity[0:B, 0:B])
                nc.scalar.copy(hpslT_sb[:], pt3[:])

                # GRU weights (pre-transposed on host): [H, 3*HL]
                wihT_sb = p1.tile([128, 8 * 3 * HL], f32)
                nc.sync.dma_start(
                    wihT_sb[:], wihT_l.rearrange("(ic p) j -> p ic j", p=128))
                whhT_sb = p1.tile([128, 8 * 3 * HL], f32)
                nc.sync.dma_start(
                    whhT_sb[:], whhT_l.rearrange("(ic p) j -> p ic j", p=128))
                bih_sb = p1.tile([HL, 3], f32)
                nc.sync.dma_start(bih_sb[:], bihT_l[:])
                bhh_sb = p1.tile([HL, 3], f32)
                nc.sync.dma_start(bhh_sb[:], bhhT_l[:])
                bsum_sb = p1.tile([HL, 3], f32)
                nc.vector.tensor_add(bsum_sb[:], bih_sb[:], bhh_sb[:])

                # gates: [128, 4*B] psum: r | z | n_ih | n_hh
                gru_ps = p1ps.tile([128, 4 * B], f32)
                J = 3 * HL
                for g, use_hh, col in ((0, True, 0), (1, True, 1), (2, False, 2),
                                       (2, None, 3)):
                    # col 0/1: accumulate ih then hh; col 2: ih only; col 3: hh only
                    o = gru_ps[:, col * B:(col + 1) * B]
                    if use_hh is None:      # n gate, hh part
                        for ic in range(8):
                            nc.tensor.matmul(
                                o, whhT_sb[:, ic * J + g * HL: ic * J + (g + 1) * HL],
                                hpT_sb[:, ic * B:(ic + 1) * B],
                                start=(ic == 0), stop=(ic == 7))
                    elif not use_hh:        # n gate, ih part
                        for ic in range(8):
                            nc.tensor.matmul(
                                o, wihT_sb[:, ic * J + g * HL: ic * J + (g + 1) * HL],
                                xT_sb[:, ic * B:(ic + 1) * B],
                                start=(ic == 0), stop=(ic == 7))
                    else:                   # r/z: ih + hh accumulated together
                        for ic in range(8):
                            nc.tensor.matmul(
                                o, wihT_sb[:, ic * J + g * HL: ic * J + (g + 1) * HL],
                                xT_sb[:, ic * B:(ic + 1) * B],
                                start=(ic == 0), stop=False)
                        for ic in range(8):
                            nc.tensor.matmul(
                                o, whhT_sb[:, ic * J + g * HL: ic * J + (g + 1) * HL],
                                hpT_sb[:, ic * B:(ic + 1) * B],
                                start=False, stop=(ic == 7))

                r_sb = p1.tile([128, B], f32)
                nc.scalar.activation(r_sb[:], gru_ps[:, 0:B], AF.Sigmoid,
                                     bias=bsum_sb[:, 0:1])
                z_sb = p1.tile([128, B], f32)
                nc.scalar.activation(z_sb[:], gru_ps[:, B:2 * B], AF.Sigmoid,
                                     bias=bsum_sb[:, 1:2])
                ghn_sb = p1.tile([128, B], f32)
                nc.scalar.activation(ghn_sb[:], gru_ps[:, 3 * B:4 * B], AF.Identity,
                                     bias=bhh_sb[:, 2:3])
                rg_sb = p1.tile([128, B], f32)
                nc.vector.tensor_mul(rg_sb[:], r_sb[:], ghn_sb[:])
                npre_sb = p1.tile([128, B], f32)
                nc.vector.tensor_add(npre_sb[:], gru_ps[:, 2 * B:3 * B], rg_sb[:])
                n_sb = p1.tile([128, B], f32)
                nc.scalar.activation(n_sb[:], npre_sb[:], AF.Tanh,
                                     bias=bih_sb[:, 2:3])
                # h_newT = n + z*(h_prev - n)
                d_sb = p1.tile([128, B], f32)
                nc.vector.tensor_tensor(d_sb[:], hpslT_sb[:], n_sb[:],
                                        op=OP.subtract)
                zd_sb = p1.tile([128, B], f32)
                nc.vector.tensor_mul(zd_sb[:], z_sb[:], d_sb[:])
                hnT_sl = p1.tile([128, B], f32)
                nc.vector.tensor_add(hnT_sl[:], n_sb[:], zd_sb[:])

                # h_new output slice (back to [B, HL])
                pt4 = tps.tile([B, 128], f32, tag="tpo")
                nc.tensor.transpose(pt4[:], hnT_sl[:], identity[:, :])
                hn_out_sb = p1.tile([B, HL], f32)
                nc.scalar.copy(hn_out_sb[:], pt4[:])
                nc.sync.dma_start(hnew_l[:], hn_out_sb[:])

                # AllToAll: trade my h-slice (all batches) for all h (my batches)
                a2a_in = dramp.tile([N, 128, BL], f32)
                a2a_out = dramp.tile([N, 128, BL], f32)
                nc.sync.dma_start(
                    a2a_in.rearrange("j p b -> p j b"), hnT_sl[:])
                nc.gpsimd.collective_compute(
                    "AllToAll", OP.bypass, replica_groups=rg,
                    ins=[a2a_in.opt()], outs=[a2a_out.opt()])
                hnT_sb = cst.tile([128, 8 * BL], f32)   # [128, 64]: (gc, my b)
                nc.sync.dma_start(
                    hnT_sb[:], a2a_out.rearrange("r p b -> p r b"))

                # q^T = attn_w^T-contraction:  qT[h, b] = sum_g attn_w[g,h] hnT[g,b]
                attn_sb = p1.tile([128, 8 * H], f32)
                nc.sync.dma_start(
                    attn_sb[:], attn_wt.rearrange("(gc p) h -> p gc h", p=128))
                q_rows = cst.tile([BL, H], f32)
                for hc in range(8):
                    q_ps = tps.tile([128, BL], f32, tag="qp")
                    for gc in range(8):
                        nc.tensor.matmul(
                            q_ps[:],
                            attn_sb[:, gc * H + hc * 128: gc * H + (hc + 1) * 128],
                            hnT_sb[:, gc * BL:(gc + 1) * BL],
                            start=(gc == 0), stop=(gc == 7))
                    qc_sb = p1.tile([128, BL], f32, tag="qc")
                    nc.scalar.copy(qc_sb[:], q_ps[:])
                    qt_ps = tps.tile([BL, 128], f32, tag="qtp")
                    nc.tensor.transpose(qt_ps[:], qc_sb[:], identity[:, :])
                    nc.scalar.copy(q_rows[:, hc * 128:(hc + 1) * 128], qt_ps[:])

            # ================== Phase 2: attention stream ================
            scoresT = cst.tile([128, SC * BL], f32)   # [128, (sc, b)]
            expT = cst.tile([128, SC * BL], f32)
            with (
                tc.tile_pool(name="enc", bufs=6) as encp,
                tc.tile_pool(name="scr", bufs=2) as scrp,
                tc.tile_pool(name="qb", bufs=2) as qbp,
                tc.tile_pool(name="stps", bufs=1, space="PSUM") as stps,
                tc.tile_pool(name="ctps", bufs=2, space="PSUM") as ctps,
                tc.tile_pool(name="w2", bufs=1) as w2p,
                tc.tile_pool(name="outw", bufs=26) as outwp,
            ):
                # context accumulates in SBUF via DVE adds: interleaved
                # multi-matmul PSUM accumulation groups misaccumulate on HW
                ctx_acc = cst.tile([128, 8 * BL], f32)   # [128, (hc, b)]
                nc.vector.memset(ctx_acc[:], 0.0)
                sink = scrp.tile([128, H], f32, tag="sink")
                # out_w^T tiles prefetched during the stream
                ow_tiles = {}
                ow_order = [(hc, vc) for vc in range(VL // VC) for hc in range(8)]
                ow_iter = iter(ow_order)

                def prefetch_ow(k=1):
                    for _ in range(k):
                        key = next(ow_iter, None)
                        if key is None:
                            return
                        hc, vc = key
                        t = outwp.tile([128, VC], f32, tag="ow")
                        nc.sync.dma_start(
                            t[:], out_wT_l[hc * 128:(hc + 1) * 128,
                                           vc * VC:(vc + 1) * VC])
                        ow_tiles[key] = t

                concat_w_sb = w2p.tile([128, 16 * H], f32)
                nc.sync.dma_start(
                    concat_w_sb[:],
                    concat_wT.rearrange("(ci p) h -> p ci h", p=128))

                for b in range(BL):
                    # engine APs can't start at partition b>0; DMA-stage the
                    # q row to partition 0 first (DMAs have no such limit)
                    qrow = qbp.tile([1, H], f32, tag="qrow")
                    nc.sync.dma_start(qrow[:], q_rows[b:b + 1, :])
                    qb = qbp.tile([128, H], f32, tag="qb")
                    nc.gpsimd.partition_broadcast(qb[:], qrow[0:1, :])
                    for sc in range(SC):
                        et = encp.tile([128, H], f32, tag="enc")
                        nc.sync.dma_start(
                            et[:], enc_l[sc * 128:(sc + 1) * 128, b, :])
                        col = sc * BL + b
                        # scores: DVE does the elementwise q*enc product, the
                        # scalar engine reduces it over h via accum_out
                        # (tensor_tensor_reduce faults at runtime on this HW)
                        scratch = scrp.tile([128, H], f32, tag="scr")
                        nc.vector.tensor_mul(scratch[:], et[:], qb[:])
                        nc.scalar.activation(sink[:], scratch[:], AF.Copy,
                                             accum_out=scoresT[:, col:col + 1])
                        nc.scalar.activation(expT[:, col:col + 1],
                                             scoresT[:, col:col + 1], AF.Exp)
                        ctx_ps = ctps.tile([128, 8], f32, tag="ctxp")
                        for hc in range(8):
                            nc.tensor.matmul(
                                ctx_ps[:, hc:hc + 1],
                                et[:, hc * 128:(hc + 1) * 128],
                                expT[:, col:col + 1],
                                start=True, stop=True)
                        ctx_view = ctx_acc[:, :].rearrange(
                            "p (hc b) -> p hc b", hc=8)[:, :, b]
                        nc.vector.tensor_add(ctx_view, ctx_view, ctx_ps[:])
                        prefetch_ow(1 if (sc % 2 == 0) else 0)

                # softmax denominators: sum exp over s for each b
                sums_ps = stps.tile([1, SC * BL], f32, tag="sums")
                nc.tensor.matmul(sums_ps[:], ones_col[:], expT[:],
                                 start=True, stop=True)
                sums_sb = cst.tile([1, SC * BL], f32)
                nc.scalar.copy(sums_sb[:], sums_ps[:])
                sumb_sb = cst.tile([1, BL], f32)
                nc.vector.reduce_sum(
                    sumb_sb[:],
                    sums_sb.rearrange("p (sc b) -> p b sc", sc=SC),
                    axis=mybir.AxisListType.X)
                recip_sb = cst.tile([1, BL], f32)
                nc.vector.reciprocal(recip_sb[:], sumb_sb[:])
                recip_bc = cst.tile([128, BL], f32)
                nc.gpsimd.partition_broadcast(recip_bc[:], recip_sb[0:1, :])

                # attention weights output: w = exp * recip[b], transposed to [b, s]
                wT_sb = cst.tile([128, SC * BL], f32)
                for sc in range(SC):
                    nc.vector.tensor_mul(wT_sb[:, sc * BL:(sc + 1) * BL],
                                         expT[:, sc * BL:(sc + 1) * BL],
                                         recip_bc[:])
                wt_ps = stps.tile([128, 128], f32, tag="wt")
                nc.tensor.transpose(wt_ps[:], wT_sb[:], identity[:, :])
                w_cb = cst.tile([128, 128], f32)
                nc.scalar.copy(w_cb[:], wt_ps[:])
                nc.sync.dma_start(
                    attnw_l.rearrange("b (sc s) -> sc b s", sc=SC), w_cb[:])

                # context^T scaled: catT bottom half  [128, (hc, b)]
                catT_ctx = cst.tile([128, 8 * BL], f32)
                for hc in range(8):
                    nc.vector.tensor_mul(catT_ctx[:, hc * BL:(hc + 1) * BL],
                                         ctx_acc[:, hc * BL:(hc + 1) * BL],
                                         recip_bc[:])

                # =============== Phase 3: concat + out proj ==============
                with tc.tile_pool(name="p3ps", bufs=1, space="PSUM") as p3ps:
                    cb_sb = w2p.tile([1, H], f32)
                    nc.sync.dma_start(cb_sb[:], concat_b_r[:])
                    co_sb = w2p.tile([BL, H], f32)
                    for hh in range(2):
                        co_ps = p3ps.tile([BL, 512], f32, tag="co")
                        for ci in range(16):
                            lhs = (hnT_sb[:, ci * BL:(ci + 1) * BL] if ci < 8
                                   else catT_ctx[:, (ci - 8) * BL:(ci - 7) * BL])
                            nc.tensor.matmul(co_ps[:], lhs,
                                             concat_w_sb[:, ci * H + hh * 512:
                                                         ci * H + (hh + 1) * 512],
                                             start=(ci == 0), stop=False)
                        nc.tensor.matmul(
                            co_ps[:], ones_col[0:1, :].to_broadcast([1, BL]),
                            cb_sb[0:1, hh * 512:(hh + 1) * 512],
                            start=False, stop=True)
                        nc.scalar.activation(co_sb[:, hh * 512:(hh + 1) * 512],
                                             co_ps[:], AF.Tanh)

                    ag_in = dramp.tile([BL, H], f32)
                    ag_out = dramp.tile([N, BL, H], f32)
                    nc.sync.dma_start(ag_in[:], co_sb[:])
                    nc.gpsimd.collective_compute(
                        "AllGather", OP.bypass, replica_groups=rg,
                        ins=[ag_in.opt()], outs=[ag_out.opt()])
                    cof_sb = w2p.tile([B, H], f32)
                    nc.sync.dma_start(
                        cof_sb[:], ag_out.rearrange("r b h -> (r b) h"))

                    coT_sb = w2p.tile([128, 8 * B], f32)
                    for hc in range(8):
                        ct_ps = p3ps.tile([128, B], f32, tag="ct")
                        nc.tensor.transpose(ct_ps[:],
                                            cof_sb[:, hc * 128:(hc + 1) * 128],
                                            identity[0:B, 0:B])
                        nc.scalar.copy(coT_sb[:, hc * B:(hc + 1) * B], ct_ps[:])

                    if debug:
                        nc.sync.dma_start(dbg_ctx[:], catT_ctx[:])
                        nc.sync.dma_start(dbg_co[:], co_sb[:])
                        nc.sync.dma_start(dbg_cof[:], cof_sb[:])
                        nc.sync.dma_start(dbg_coT[:], coT_sb[:])

                    prefetch_ow(64)   # any tiles not yet queued
                    for vc in range(VL // VC):
                        ob_sb = w2p.tile([1, VC], f32, tag="ob")
                        nc.sync.dma_start(ob_sb[:],
                                          out_b_l[0:1, vc * VC:(vc + 1) * VC])
                        lg_ps = p3ps.tile([B, VC], f32, tag="lg")
                        for hc in range(8):
                            nc.tensor.matmul(lg_ps[:], coT_sb[:, hc * B:(hc + 1) * B],
                                             ow_tiles[(hc, vc)][:],
                                             start=(hc == 0), stop=False)
                        nc.tensor.matmul(
                            lg_ps[:], ones_col[0:1, :].to_broadcast([1, B]),
                            ob_sb[0:1, :],
                            start=False, stop=True)
                        lg_sb = w2p.tile([B, VC], f32, tag="lgs")
                        nc.scalar.copy(lg_sb[:], lg_ps[:])
                        nc.sync.dma_start(logits_l[:, vc * VC:(vc + 1) * VC],
                                          lg_sb[:])

    nc.compile()
    return nc


def _prep_in_maps(input_seq, last_hidden, encoder_outputs, emb, w_ih, w_hh,
                  b_ih, b_hh, attn_w, concat_w, concat_b, out_w, out_b):
    f = np.float32
    seq = np.asarray(input_seq).astype(np.int32).reshape(B, 1)
    hp = np.ascontiguousarray(np.asarray(last_hidden, f)[0])          # [B, H]
    enc = np.asarray(encoder_outputs, f)                              # [S, B, H]
    emb = np.asarray(emb, f)
    w_ih = np.asarray(w_ih, f)
    w_hh = np.asarray(w_hh, f)
    b_ih = np.asarray(b_ih, f)
    b_hh = np.asarray(b_hh, f)
    attn_w = np.ascontiguousarray(np.asarray(attn_w, f))
    concat_wT = np.ascontiguousarray(np.asarray(concat_w, f).T)       # [2H, H]
    concat_b_r = np.asarray(concat_b, f).reshape(1, H)
    out_wT = np.ascontiguousarray(np.asarray(out_w, f).T)             # [H, V]
    out_b = np.asarray(out_b, f)
    ident = np.eye(128, dtype=f)

    in_maps = []
    for k in range(N):
        rows = [slice(g * H + k * HL, g * H + (k + 1) * HL) for g in range(3)]
        wihT_l = np.ascontiguousarray(
            np.concatenate([w_ih[r] for r in rows], axis=0).T)        # [H, 3HL]
        whhT_l = np.ascontiguousarray(
            np.concatenate([w_hh[r] for r in rows], axis=0).T)
        bihT_l = np.ascontiguousarray(
            np.stack([b_ih[r] for r in rows], axis=1))                # [HL, 3]
        bhhT_l = np.ascontiguousarray(
            np.stack([b_hh[r] for r in rows], axis=1))
        in_maps.append({
            "seq_idx": seq,
            "emb": emb,
            "hprev": hp,
            "hprev_sl": np.ascontiguousarray(hp[:, k * HL:(k + 1) * HL]),
            "enc_l": np.ascontiguousarray(enc[:, k * BL:(k + 1) * BL, :]),
            "wihT_l": wihT_l,
            "whhT_l": whhT_l,
            "bihT_l": bihT_l,
            "bhhT_l": bhhT_l,
            "attn_w": attn_w,
            "concat_wT": concat_wT,
            "concat_b_r": concat_b_r,
            "out_wT_l": np.ascontiguousarray(out_wT[:, k * VL:(k + 1) * VL]),
            "out_b_l": out_b[k * VL:(k + 1) * VL].reshape(1, VL),
            "ident": ident,
        })
    return in_maps


def kernel(input_seq, last_hidden, encoder_outputs, emb, w_ih, w_hh, b_ih,
           b_hh, attn_w, attn_b, concat_w, concat_b, out_w, out_b):
    # attn_b is intentionally unused: scores only ever enter a softmax over s,
    # and the attn_b contribution (h_new . attn_b) is constant over s, so it
    # cancels exactly.
    from concourse import bass_utils

    if "nc" not in _cache:
        _cache["nc"] = _build()
    nc = _cache["nc"]

    in_maps = _prep_in_maps(input_seq, last_hidden, encoder_outputs, emb,
                            w_ih, w_hh, b_ih, b_hh, attn_w, concat_w,
                            concat_b, out_w, out_b)

    trace = bool(os.environ.get("ATTN_KERNEL_TRACE"))
    res = bass_utils.run_bass_kernel_spmd(
        nc, in_maps, core_ids=list(range(N)), trace=trace)
    if trace:
        _cache["exec_time_ns"] = res.exec_time_ns
    _cache["results"] = res.results

    logits = np.concatenate([res.results[k]["logits_l"] for k in range(N)],
                            axis=1)
    h_new = np.concatenate([res.results[k]["hnew_l"] for k in range(N)],
                           axis=1)[None]
    attnw = np.concatenate([res.results[k]["attnw_l"] for k in range(N)],
                           axis=0)[:, None, :]
    return logits, h_new, attnw
